# revision 1
# baseline (speedup 1.0000x reference)
"""Masked attention (B=4, M=N=4096, D=64) on 8 Trainium2 NeuronCores.

Sharding: batch (4) x m-halves (2) -> 8 cores, no cross-core communication.
Each core computes out[m, :] = softmax(mask(q@k^T)/sqrt(d)) @ v for its
2048 q rows against the full 4096 k/v rows of its batch.

Device algorithm (per core), designed around engine rooflines:
  - Scores are computed TRANSPOSED: S^T[n, m] = (kT chunk).T @ qT, so the
    attention-weight matrix is produced with n (the PV contraction dim) on
    partitions -- no transpose of the big attention matrix is ever needed.
    q and k are shipped pre-transposed ([d, m] / [d, n]) in fp16 from the
    host (fp16 keeps matmuls at 1 cycle/row with ~10x better precision
    than bf16 for N(0,1) data).
  - ScalarE computes e = exp(S^T * 1/sqrt(d) - 3) straight out of PSUM
    (the -3 shift cancels in softmax normalization); this 1-elem/cycle
    stream is the pacing engine of the whole kernel.
  - VectorE applies the mask: p = e * notmaskT (fp16 tensor_tensor, 2x).
  - PV: out^T[j, m] += v_aug_chunk.T @ p, where v_aug = [v | ones]; the
    ones column makes row 64 of out^T the softmax denominator l[m] free.
  - PV is software-pipelined one n-chunk-pair behind QK so the PE never
    stalls on the current pair's exp/mask chain.
  - The out^T [65, 1024] accumulators ship raw to the host, which does the
    (tiny) divide-by-l + transpose while unsharding.
  - QK matmuls have K=64 (= d), so consecutive n-chunks are packed into
    row-halves of the PE array (tile_position via base_partition 0/64).
  - Dense K=128 "keepalive" matmul bursts pin the PE HAM clock-gate at
    8/8 (2.4 GHz); without them the whole kernel runs at 1.2 GHz.
"""

import numpy as np
import ml_dtypes
from contextlib import ExitStack

import concourse.bacc as bacc
import concourse.mybir as mybir
import concourse.tile as tile
from concourse.bass_utils import run_bass_kernel_spmd

B, M, N, D = 4, 4096, 4096, 64
NCORES = 8
M_LOC = M // 2        # q rows per core
MH = 1024             # m sub-block held in one PSUM accumulation
NCH = N // 128        # 32 n-chunks of 128
SCALE = 1.0 / 8.0     # 1/sqrt(64)
EBIAS = -3.0
BF16 = mybir.dt.bfloat16
F32 = mybir.dt.float32
F32R = mybir.dt.float32r
FP16 = mybir.dt.float16
BF = ml_dtypes.bfloat16

_NC = None
LAST_RESULTS = None   # BassKernelResults of the most recent run (for profiling)
TRACE = False
TRACE_KW = {}


def _build_nc():
    nc = bacc.Bacc("TRN2", target_bir_lowering=False, debug=False,
                   num_devices=NCORES)
    qT = nc.dram_tensor("qT", [128, M_LOC], FP16, kind="ExternalInput").ap()
    kT = nc.dram_tensor("kT", [128, (NCH // 2) * 128], FP16,
                        kind="ExternalInput").ap()
    vA = nc.dram_tensor("vA", [128, NCH * (D + 1)], FP16,
                        kind="ExternalInput").ap()
    nmT = nc.dram_tensor("nmT", [N, M_LOC], FP16, kind="ExternalInput").ap()
    # raw accumulator output: out^T with the softmax denominator in row 64;
    # the host does the (tiny) divide + transpose during unsharding
    o = nc.dram_tensor("oT", [2, D + 1, MH], F32, kind="ExternalOutput").ap()

    with tile.TileContext(nc) as tc, ExitStack() as ctx:
        const = ctx.enter_context(tc.tile_pool(name="const", bufs=1))
        mpool = ctx.enter_context(tc.tile_pool(name="mask", bufs=6))
        epool = ctx.enter_context(tc.tile_pool(name="e", bufs=4))
        ppool = ctx.enter_context(tc.tile_pool(name="p", bufs=4))
        fpool = ctx.enter_context(tc.tile_pool(name="fin", bufs=2))
        spool = ctx.enter_context(tc.tile_pool(name="spsum", bufs=2, space="PSUM"))
        opool = ctx.enter_context(tc.tile_pool(name="opsum", bufs=1, space="PSUM"))
        wpool = ctx.enter_context(tc.tile_pool(name="wpsum", bufs=1, space="PSUM"))

        # spread the constant loads over three DMA queues so they overlap
        qT_s = const.tile([128, M_LOC], FP16)
        nc.sync.dma_start(qT_s[:], qT)
        kT_s = const.tile([128, (NCH // 2) * 128], FP16)
        nc.scalar.dma_start(kT_s[:], kT)
        vA_s = const.tile([128, NCH * (D + 1)], FP16)
        nc.scalar.dma_start(vA_s[:], vA)
        ebias = const.tile([128, 1], F32)
        nc.vector.memset(ebias[:], EBIAS)
        # warmup operand with no DMA dependency (starts right after preamble)
        wsrc = const.tile([128, 512], BF16)
        nc.vector.memset(wsrc[:], 1.0)

        # Dense back-to-back full-array (K=128) matmuls keep the PE HAM
        # clock-gate at 8/8 (results discarded). One burst before each
        # m-half: the first warms the PE up, the second bridges the PE-idle
        # window during the h=0 finalize so the gate never re-throttles.
        # (K=64 matmuls do NOT trip the activity monitor.)
        wu = wpool.tile([128, 512], F32)

        def pe_keepalive(n):
            for _ in range(n):
                nc.tensor.matmul(wu[:], wsrc[:, 0:128], wsrc[:, 0:512],
                                 start=True, stop=True)

        for h in range(2):
            if h == 0:
                pe_keepalive(10)
            o_ps = opool.tile([D + 1, MH], F32)
            pv_pending = []

            def flush_pv():
                for ni, p in pv_pending:
                    vch = vA_s[:, ni * (D + 1):(ni + 1) * (D + 1)]
                    nc.tensor.matmul(o_ps[:, 0:512], vch, p[:, 0:512],
                                     start=(ni == 0), stop=(ni == NCH - 1))
                    nc.tensor.matmul(o_ps[:, 512:1024], vch, p[:, 512:1024],
                                     start=(ni == 0), stop=(ni == NCH - 1))
                pv_pending.clear()

            for pc in range(NCH // 2):
                ni_e, ni_o = 2 * pc, 2 * pc + 1
                lhs_e = kT_s[0:64, pc * 128:(pc + 1) * 128]
                lhs_o = kT_s[64:128, pc * 128:(pc + 1) * 128]
                rhs_e = qT_s[0:64, h * MH:(h + 1) * MH]
                rhs_o = qT_s[64:128, h * MH:(h + 1) * MH]
                S_e = spool.tile([128, MH], F32, tag="s")
                S_o = spool.tile([128, MH], F32, tag="s")
                # interleave row-halves so consecutive MMs overlap on the PE
                nc.tensor.matmul(S_e[:, 0:512], lhs_e, rhs_e[:, 0:512],
                                 start=True, stop=True)
                nc.tensor.matmul(S_o[:, 0:512], lhs_o, rhs_o[:, 0:512],
                                 start=True, stop=True)
                nc.tensor.matmul(S_e[:, 512:1024], lhs_e, rhs_e[:, 512:1024],
                                 start=True, stop=True)
                nc.tensor.matmul(S_o[:, 512:1024], lhs_o, rhs_o[:, 512:1024],
                                 start=True, stop=True)
                # at the half boundary the keepalive comes AFTER the first
                # QK pair so the exp stream restarts without a gap
                if h == 1 and pc == 0:
                    pe_keepalive(12)
                # PV of the PREVIOUS pair goes right after this pair's QK so
                # the PE never waits on the current pair's exp/mask chain.
                flush_pv()
                # one DMA for the pair's mask rows: [256, MH] -> [128, 2*MH]
                nm = mpool.tile([128, 2 * MH], FP16)
                nm_src = nmT[ni_e * 128:(ni_e + 2) * 128,
                             h * MH:(h + 1) * MH].rearrange(
                                 "(t p) m -> p t m", t=2)
                nc.sync.dma_start(nm[:].rearrange("p (t m) -> p t m", t=2),
                                  nm_src)
                for half, (ni, S) in enumerate(((ni_e, S_e), (ni_o, S_o))):
                    e = epool.tile([128, MH], FP16)
                    nc.scalar.activation(e[:], S[:],
                                         mybir.ActivationFunctionType.Exp,
                                         bias=ebias[:], scale=SCALE)
                    p = ppool.tile([128, MH], FP16)
                    nc.vector.tensor_mul(p[:], e[:],
                                         nm[:, half * MH:(half + 1) * MH])
                    pv_pending.append((ni, p))
            flush_pv()
            oT = fpool.tile([D + 1, MH], F32)
            nc.vector.tensor_copy(oT[:, 0:MH // 2], o_ps[:, 0:MH // 2])
            nc.sync.dma_start(o[h, :, 0:MH // 2], oT[:, 0:MH // 2])
            nc.vector.tensor_copy(oT[:, MH // 2:MH], o_ps[:, MH // 2:MH])
            nc.sync.dma_start(o[h, :, MH // 2:MH], oT[:, MH // 2:MH])
    nc.compile()
    return nc


def _get_nc():
    global _NC
    if _NC is None:
        _NC = _build_nc()
    return _NC


def _prep_core(q, k, v, mask, b, j):
    qs = q[b, j * M_LOC:(j + 1) * M_LOC, :]
    qT = np.ascontiguousarray(qs.T).astype(np.float16)    # [64, 2048]
    qTp = np.concatenate([qT, qT], axis=0)                # [128, 2048]
    kTf = np.ascontiguousarray(k[b].T).astype(np.float16) # [64, 4096]
    kTp = np.empty((128, (NCH // 2) * 128), np.float16)
    kTr = kTf.reshape(64, NCH, 128)
    kTp[0:64] = kTr[:, 0::2, :].reshape(64, -1)
    kTp[64:128] = kTr[:, 1::2, :].reshape(64, -1)
    vb = v[b]                                             # [4096, 64]
    vA = np.empty((128, NCH * (D + 1)), np.float16)
    vAr = vA.reshape(128, NCH, D + 1)
    vAr[:, :, :D] = vb.reshape(NCH, 128, D).transpose(1, 0, 2).astype(np.float16)
    vAr[:, :, D] = np.float16(1.0)
    nmT = np.ascontiguousarray(
        (~mask[b, j * M_LOC:(j + 1) * M_LOC, :]).T).astype(np.float16)
    return {"qT": qTp, "kT": kTp, "vA": vA, "nmT": nmT}


def kernel(q, k, v, mask):
    global LAST_RESULTS
    q = np.asarray(q, dtype=np.float32)
    k = np.asarray(k, dtype=np.float32)
    v = np.asarray(v, dtype=np.float32)
    mask = np.asarray(mask)
    nc = _get_nc()
    in_maps = [_prep_core(q, k, v, mask, c // 2, c % 2) for c in range(NCORES)]
    res = run_bass_kernel_spmd(nc, in_maps, core_ids=list(range(NCORES)),
                               trace=TRACE, **TRACE_KW)
    LAST_RESULTS = res
    out = np.empty((B, M, D), np.float32)
    for c in range(NCORES):
        b, j = divmod(c, 2)
        oT = res.results[c]["oT"]                      # [2, 65, MH]
        for h in range(2):
            blk = oT[h, :D, :] / oT[h, D, :]           # [64, MH]
            lo = j * M_LOC + h * MH
            out[b, lo:lo + MH, :] = blk.T
    return out



# revision 2
# speedup vs baseline: 1.1983x; 1.1983x over previous
"""Masked attention (B=4, M=N=4096, D=64) on 8 Trainium2 NeuronCores.

Sharding: batch (4) x m-halves (2) -> 8 cores, no cross-core communication.
Each core computes out[m, :] = softmax(mask(q@k^T)/sqrt(d)) @ v for its
2048 q rows against the full 4096 k/v rows of its batch.

v2 device algorithm (per core), engine-balanced around the PE roofline:
  - q/k are host-scaled by sqrt(1024*log2e/8) so the QK matmul directly
    produces S = 1024*log2(e^(s)) "fp16 exponent bits" units in PSUM f32.
  - Scores are computed TRANSPOSED per 128-row n-chunk (S^T[n, m]) so the
    attention weights come out with n (the PV contraction dim) on partitions.
  - exp+mask runs through two routes, load-balancing ScalarE and VectorE:
    * ACT route (12/16 pairs): ScalarE e = Exp(S/1477.12 + 3.46574) fp16,
      then VectorE p = e * nm (multiplicative fp16 {0,1} mask, 2x mode).
    * DVE route (4/16 pairs): one custom VectorE op computes
      bits_u16 = S + mb + |frac1024(S+mb)| * (1024-|..|) * (gamma/1024),
      i.e. a quadratic-mantissa-corrected 2^x in fp16 BIT SPACE (rel err
      ~0.2%), with the softmax bias AND the mask folded into the additive
      fp8e5 operand mb in {12288 (masked, 2^-8 suppression), 20480}. The
      uint16 result IS the fp16 attention weight (reinterpreted view) --
      exp AND mask in a single 1-byte-mask DVE pass, no ScalarE work.
  - PV: out^T[j, m] += v_aug_chunk.T @ p (v_aug = [v | ones]; row 64 of
    out^T = softmax denominator for free). PV is software-pipelined one
    pair behind QK.
  - The out^T [65, 1024] accumulators ship raw; host divides/transposes.
  - Dense K=128 keepalive matmul bursts pin the PE HAM clock-gate at 8/8
    (2.4 GHz); K=64 QK matmuls do not trip the activity monitor.
"""

import numpy as np
import ml_dtypes
from contextlib import ExitStack

import concourse.bacc as bacc
import concourse.mybir as mybir
import concourse.tile as tile
from concourse.bass_utils import run_bass_kernel_spmd

B, M, N, D = 4, 4096, 4096, 64
NCORES = 8
M_LOC = M // 2        # q rows per core
MH = 1024             # m sub-block held in one PSUM accumulation
NCH = N // 128        # 32 n-chunks of 128
NPAIR = NCH // 2      # 16 chunk-pairs
DVE_PAIRS = (2, 6, 10, 14)   # pairs routed to the custom DVE exp2 op
ACT_PAIRS = tuple(p for p in range(NPAIR) if p not in DVE_PAIRS)

LOG2E = float(np.log2(np.e))
SQ = float(np.sqrt(1024.0 * LOG2E / 8.0))   # 13.5874: q/k pre-scale
MAGIC = 3.0 * 2.0**32                        # RNE-to-multiple-of-1024 magic
B_MASK, B_KEEP = 12288.0, 20480.0            # additive fp8e5 mask biases
GAMMA = -0.346                               # quadratic mantissa correction
ACT_SCALE = 1.0 / (1024.0 * LOG2E)
ACT_BIAS = (B_KEEP - 15360.0) / (1024.0 * LOG2E)   # +3.46574 (matches B_KEEP)

BF16 = mybir.dt.bfloat16
F32 = mybir.dt.float32
FP16 = mybir.dt.float16
FP8E5 = mybir.dt.float8e5
U16 = mybir.dt.uint16
E5 = ml_dtypes.float8_e5m2

_NC = None
_EXP2_OP = None
LAST_RESULTS = None   # BassKernelResults of the most recent run (for profiling)
TRACE = False
TRACE_KW = {}


def _get_exp2_op():
    """Register (once) the corrected-exp2 custom DVE op: out_u16 =
    t + |t - rne1024(t)| * (1024 - |..|) * (gamma/1024), t = in0 + in1."""
    global _EXP2_OP
    if _EXP2_OP is not None:
        return _EXP2_OP
    from concourse.dve_spec import C0, C1, C2, AluOp, Bin, Spec, Src0, Src1, lower
    from concourse.dve_spec import _has_src1
    from concourse.dve_uop import DveOpSpec
    from concourse import dve_ops as dops

    t = Src0 + Src1
    r = (t + C0) - C0
    d = Bin(AluOp.ABSOLUTE_DIFF, t, r)
    body = t + (d * (C1 - d)) * C2

    def ref(in0, in1, s0, s1, imm2):
        tt = (in0.astype(np.float32) + in1.astype(np.float32)).astype(np.float32)
        rr = ((tt + np.float32(s0)).astype(np.float32) - np.float32(s0)).astype(
            np.float32)
        dd = np.abs((tt - rr).astype(np.float32))
        return (tt + dd * (np.float32(s1) - dd) * np.float32(imm2)).astype(
            np.float32)

    spec = Spec(body=body, reference=ref)
    name = "EXP2_BITS_ANT"
    if name not in dops._SUB_OPCODE_FOR_NAME:
        row = max(dops._SUB_OPCODE_FOR_NAME.values()) + 1
        assert row < 0x20
        dops._SUB_OPCODE_FOR_NAME[name] = row
        sha = DveOpSpec(name=name, opcode=row, uops=lower(spec, ver="v3"),
                        rd1_en=_has_src1(spec)).sha("v3")
        op = dops.DveOp(name=name, spec=spec, subdim=False,
                        uops_sha={"v3": sha})
        dops.OPS.append(op)
        dops.CUSTOM_DVE_SPECS[name] = spec
        _EXP2_OP = op
    else:
        _EXP2_OP = next(o for o in dops.OPS if o.name == name)
    return _EXP2_OP


def _build_nc():
    exp2_op = _get_exp2_op()
    nc = bacc.Bacc("TRN2", target_bir_lowering=False, debug=False,
                   num_devices=NCORES)
    qT = nc.dram_tensor("qT", [64, M_LOC], FP16, kind="ExternalInput").ap()
    kT = nc.dram_tensor("kT", [64, N], FP16, kind="ExternalInput").ap()
    vA = nc.dram_tensor("vA", [128, NCH * (D + 1)], FP16,
                        kind="ExternalInput").ap()
    nmT16 = nc.dram_tensor("nmT16", [len(ACT_PAIRS) * 256, M_LOC], FP16,
                           kind="ExternalInput").ap()
    nmB8 = nc.dram_tensor("nmB8", [len(DVE_PAIRS) * 256, M_LOC], FP8E5,
                          kind="ExternalInput").ap()
    # raw accumulator output: out^T with the softmax denominator in row 64
    o = nc.dram_tensor("oT", [2, D + 1, MH], F32, kind="ExternalOutput").ap()

    with tile.TileContext(nc) as tc, ExitStack() as ctx:
        const = ctx.enter_context(tc.tile_pool(name="const", bufs=1))
        mpool = ctx.enter_context(tc.tile_pool(name="mask16", bufs=6))
        bpool = ctx.enter_context(tc.tile_pool(name="mask8", bufs=4))
        epool = ctx.enter_context(tc.tile_pool(name="e", bufs=4))
        ppool = ctx.enter_context(tc.tile_pool(name="p", bufs=6))
        fpool = ctx.enter_context(tc.tile_pool(name="fin", bufs=2))
        spool = ctx.enter_context(tc.tile_pool(name="spsum", bufs=3, space="PSUM"))
        opool = ctx.enter_context(tc.tile_pool(name="opsum", bufs=1, space="PSUM"))

        # spread the constant loads over DMA queues so they overlap
        qT_s = const.tile([64, M_LOC], FP16)
        nc.sync.dma_start(qT_s[:], qT)
        kT_s = const.tile([64, N], FP16)
        nc.scalar.dma_start(kT_s[:], kT)
        vA_s = const.tile([128, NCH * (D + 1)], FP16)
        nc.scalar.dma_start(vA_s[:], vA)
        ebias = const.tile([128, 1], F32)
        nc.vector.memset(ebias[:], ACT_BIAS)
        # warmup operand with no DMA dependency (starts right after preamble)
        wsrc = const.tile([128, 512], BF16)
        nc.vector.memset(wsrc[:], 1.0)

        # Dense back-to-back full-array (K=128) matmuls keep the PE HAM
        # clock-gate at 8/8 (results discarded into a rotating S slot).
        def pe_keepalive(n):
            wu = spool.tile([128, MH], F32, tag="s")
            for _ in range(n):
                nc.tensor.matmul(wu[:, 0:512], wsrc[:, 0:128], wsrc[:, 0:512],
                                 start=True, stop=True)

        for h in range(2):
            if h == 0:
                pe_keepalive(10)
            o_ps = opool.tile([D + 1, MH], F32)
            pv_pending = []

            def flush_pv():
                for ni, pap in pv_pending:
                    vch = vA_s[:, ni * (D + 1):(ni + 1) * (D + 1)]
                    nc.tensor.matmul(o_ps[:, 0:512], vch, pap[:, 0:512],
                                     start=(ni == 0), stop=(ni == NCH - 1))
                    nc.tensor.matmul(o_ps[:, 512:1024], vch, pap[:, 512:1024],
                                     start=(ni == 0), stop=(ni == NCH - 1))
                pv_pending.clear()

            aj = dj = 0
            for pc in range(NPAIR):
                c0, c1 = 2 * pc, 2 * pc + 1
                lhs0 = kT_s[:, c0 * 128:(c0 + 1) * 128]
                lhs1 = kT_s[:, c1 * 128:(c1 + 1) * 128]
                rhs = qT_s[:, h * MH:(h + 1) * MH]
                S0 = spool.tile([128, MH], F32, tag="s")
                S1 = spool.tile([128, MH], F32, tag="s")
                # same-stationary matmuls adjacent: 1 weight load per chunk
                nc.tensor.matmul(S0[:, 0:512], lhs0, rhs[:, 0:512],
                                 start=True, stop=True)
                nc.tensor.matmul(S0[:, 512:1024], lhs0, rhs[:, 512:1024],
                                 start=True, stop=True)
                nc.tensor.matmul(S1[:, 0:512], lhs1, rhs[:, 0:512],
                                 start=True, stop=True)
                nc.tensor.matmul(S1[:, 512:1024], lhs1, rhs[:, 512:1024],
                                 start=True, stop=True)
                # at the half boundary the keepalive comes AFTER the first
                # QK pair so the exp stream restarts without a gap
                if h == 1 and pc == 0:
                    pe_keepalive(12)
                # PV of the PREVIOUS pair goes right after this pair's QK so
                # the PE never waits on the current pair's exp/mask chain.
                flush_pv()
                if pc in DVE_PAIRS:
                    nm8 = bpool.tile([128, 2 * MH], FP8E5)
                    src = nmB8[dj * 256:(dj + 1) * 256,
                               h * MH:(h + 1) * MH].rearrange(
                                   "(t p) m -> p t m", t=2)
                    nc.sync.dma_start(
                        nm8[:].rearrange("p (t m) -> p t m", t=2), src)
                    dj += 1
                    for half, (ni, S) in enumerate(((c0, S0), (c1, S1))):
                        pb = ppool.tile([128, MH], U16)
                        nc.vector._custom_dve(
                            exp2_op, out=pb[:], in0=S[:],
                            in1=nm8[:, half * MH:(half + 1) * MH],
                            s0=MAGIC, s1=1024.0, imm2=GAMMA / 1024.0)
                        pv_pending.append((ni, pb[:].bitcast(FP16)))
                else:
                    nm = mpool.tile([128, 2 * MH], FP16)
                    src = nmT16[aj * 256:(aj + 1) * 256,
                                h * MH:(h + 1) * MH].rearrange(
                                    "(t p) m -> p t m", t=2)
                    nc.sync.dma_start(
                        nm[:].rearrange("p (t m) -> p t m", t=2), src)
                    aj += 1
                    for half, (ni, S) in enumerate(((c0, S0), (c1, S1))):
                        e = epool.tile([128, MH], FP16)
                        nc.scalar.activation(e[:], S[:],
                                             mybir.ActivationFunctionType.Exp,
                                             bias=ebias[:], scale=ACT_SCALE)
                        p = ppool.tile([128, MH], FP16)
                        nc.vector.tensor_mul(p[:], e[:],
                                             nm[:, half * MH:(half + 1) * MH])
                        pv_pending.append((ni, p[:]))
            flush_pv()
            oT = fpool.tile([D + 1, MH], F32)
            nc.scalar.copy(oT[:, 0:MH // 2], o_ps[:, 0:MH // 2])
            nc.sync.dma_start(o[h, :, 0:MH // 2], oT[:, 0:MH // 2])
            nc.vector.tensor_copy(oT[:, MH // 2:MH], o_ps[:, MH // 2:MH])
            nc.sync.dma_start(o[h, :, MH // 2:MH], oT[:, MH // 2:MH])
    nc.compile()
    return nc


def _get_nc():
    global _NC
    if _NC is None:
        _NC = _build_nc()
    return _NC


def _prep_core(q, k, v, mask, b, j):
    qs = q[b, j * M_LOC:(j + 1) * M_LOC, :]
    qT = np.ascontiguousarray(qs.T * SQ).astype(np.float16)    # [64, 2048]
    kT = np.ascontiguousarray(k[b].T * SQ).astype(np.float16)  # [64, 4096]
    vb = v[b]                                                  # [4096, 64]
    vA = np.empty((128, NCH * (D + 1)), np.float16)
    vAr = vA.reshape(128, NCH, D + 1)
    vAr[:, :, :D] = vb.reshape(NCH, 128, D).transpose(1, 0, 2).astype(np.float16)
    vAr[:, :, D] = np.float16(1.0)
    nm = ~mask[b, j * M_LOC:(j + 1) * M_LOC, :]                # [2048, 4096]
    nmT = np.ascontiguousarray(nm.T)                           # [4096, 2048]
    a_rows = np.concatenate([nmT[2 * p * 128:(2 * p + 2) * 128]
                             for p in ACT_PAIRS], axis=0)
    nmT16 = a_rows.astype(np.float16)
    d_rows = np.concatenate([nmT[2 * p * 128:(2 * p + 2) * 128]
                             for p in DVE_PAIRS], axis=0)
    nmB8 = np.where(d_rows, np.float32(B_KEEP), np.float32(B_MASK)).astype(E5)
    return {"qT": qT, "kT": kT, "vA": vA, "nmT16": nmT16, "nmB8": nmB8}


def kernel(q, k, v, mask):
    global LAST_RESULTS
    q = np.asarray(q, dtype=np.float32)
    k = np.asarray(k, dtype=np.float32)
    v = np.asarray(v, dtype=np.float32)
    mask = np.asarray(mask)
    nc = _get_nc()
    in_maps = [_prep_core(q, k, v, mask, c // 2, c % 2) for c in range(NCORES)]
    res = run_bass_kernel_spmd(nc, in_maps, core_ids=list(range(NCORES)),
                               trace=TRACE, **TRACE_KW)
    LAST_RESULTS = res
    out = np.empty((B, M, D), np.float32)
    for c in range(NCORES):
        b, j = divmod(c, 2)
        oT = res.results[c]["oT"]                      # [2, 65, MH]
        for h in range(2):
            blk = oT[h, :D, :] / oT[h, D, :]           # [64, MH]
            lo = j * M_LOC + h * MH
            out[b, lo:lo + MH, :] = blk.T
    return out
